# revision 10
# baseline (speedup 1.0000x reference)
"""GIN 3-layer message-passing kernel for 8 Trainium2 NeuronCores.

Strategy (hardcoded for the fixed problem instance, 100k nodes / 1.6M edges /
128 feat / 1024 graphs):
  - Nodes sharded 12500/core (padded to 12544); each core owns the edges whose
    dst lands in its range (~200k), grouped by 128-node dst tile and padded to
    a per-tile count uniform across cores so one SPMD program serves all 8.
  - Each core ships only its own node slice (bf16, node-major). The full
    gather table [100352, 128] is built on device with an AllGather; the
    feature-major working copy hT comes from a one-shot DMA transpose.
  - Per-edge source rows are fetched from the table with per-128-row
    indirect DMAs (int32 indices resident in SBUF).
  - Scatter-add on TensorE: per 128-edge group, a one-hot matrix (dst-rel,
    built 3D-AP-batched by is_equal against an iota row) is the moving
    operand, the gathered rows the stationary one; PSUM accumulates
    agg^T [feat, nodes] per 128-node tile.
  - (1+eps)h + agg fused on VectorE; Linear via f32 matmul with W^T resident;
    bias+BN-stats fused into the PSUM eviction; batchnorm stats all-reduced
    across cores; normalize+ReLU as one activation with per-partition
    scale/bias.
  - Updated bf16 features exported via TensorE transpose -> DRAM ->
    AllGather back into the table for the next layer.
  - Mean-pool via one-hot matmul over a 256-graph window; host adds the 8
    partial outputs and divides by counts.
"""

import sys

sys.path.insert(0, "/opt/trn_rl_repo")

import numpy as np
import ml_dtypes

from concourse import bass, bacc, mybir
from concourse.ap import AP
from concourse.bass_utils import run_bass_kernel_spmd
from concourse.tile import TileContext

BF16 = ml_dtypes.bfloat16
LAST_RESULT = None

# ---------------------------------------------------------------- constants
CORES = 8
D = 128
N = 100000
NGR = 1024
NLOC = N // CORES            # 12500 real nodes per core
TPC = (NLOC + 127) // 128    # 98 node tiles per core
NPAD = TPC * 128             # 12544
TROWS = CORES * NPAD         # 100352 rows in the gather table
TPB = 4                      # node tiles per staging batch (PSUM budget)
NB = (TPC + TPB - 1) // TPB  # 25 batches
GWIN = 256                   # graph window for pooling (per-core span < 256)
NLAYER = 3
BN_EPS = 1e-5
SENT = 200.0                 # dstrel sentinel (never matches iota 0..127)
GSENT = 300.0                # gidrel sentinel (never matches iota 0..255)


# ---------------------------------------------------------------- host plan
def _make_plan(src, dst, batch):
    src = np.ascontiguousarray(src)
    dst = np.ascontiguousarray(dst)
    batch = np.asarray(batch)

    order = np.argsort(dst, kind="stable")
    so_src, so_dst = src[order], dst[order]
    core_starts = np.searchsorted(so_dst, np.arange(CORES + 1) * NLOC)

    # per-core per-tile counts
    cnts = np.zeros((CORES, TPC), np.int64)
    for c in range(CORES):
        lo, hi = core_starts[c], core_starts[c + 1]
        t = (so_dst[lo:hi] - c * NLOC) >> 7
        cnts[c] = np.bincount(t, minlength=TPC)

    K = ((cnts.max(axis=0) + 127) // 128) * 128  # uniform group size per tile
    goffs = np.zeros(TPC + 1, np.int64)
    np.cumsum(K, out=goffs[1:])
    NI = int(goffs[-1])
    NTILES = NI // 128

    batch_tiles = [list(range(b * TPB, min((b + 1) * TPB, TPC))) for b in range(NB)]
    # per-batch metadata: (P0, nt, [(t, pair_lo, pair_hi)])
    binfo = []
    for b in range(NB):
        ts = batch_tiles[b]
        P0 = int(goffs[ts[0]]) // 128
        nt = int(goffs[ts[-1] + 1] - goffs[ts[0]]) // 128
        mlist = [
            (t, int(goffs[t]) // 128, int(goffs[t + 1]) // 128) for t in ts
        ]
        binfo.append((P0, nt, mlist))
    BSTG = max(nt for (_, nt, _) in binfo) * 128

    # per-core data tables
    gidx_tabs, dstrel_tabs, gidrel_tabs, g_los = [], [], [], []
    for c in range(CORES):
        lo, hi = core_starts[c], core_starts[c + 1]
        es = so_src[lo:hi]
        ed = so_dst[lo:hi] - c * NLOC          # already sorted ascending
        prow = (es // NLOC) * NPAD + (es % NLOC)
        t = ed >> 7
        tile_starts = np.searchsorted(t, np.arange(TPC))
        slot = goffs[t] + (np.arange(len(ed)) - tile_starts[t])

        gidx = np.zeros(NI, np.int32)
        gidx[slot] = prow.astype(np.int32)
        drel = np.full(NI, SENT, np.float32)
        drel[slot] = (ed - (t << 7)).astype(np.float32)
        gidx_tabs.append(gidx.reshape(-1, 128).T.astype(np.int32).copy())
        dstrel_tabs.append(drel.reshape(-1, 128).T.astype(BF16).copy())

        g_lo = int(batch[c * NLOC])
        g_hi = int(batch[(c + 1) * NLOC - 1])
        assert g_hi - g_lo < GWIN, (g_lo, g_hi)
        g_los.append(g_lo)
        grel = np.full((TPC * 128,), GSENT, np.float32)
        grel[:NLOC] = (batch[c * NLOC : (c + 1) * NLOC] - g_lo).astype(np.float32)
        gidrel_tabs.append(grel.reshape(TPC, 128).T.astype(BF16).copy())

    return dict(
        binfo=binfo, NI=NI, NTILES=NTILES, BSTG=BSTG,
        gidx_tabs=gidx_tabs, dstrel_tabs=dstrel_tabs,
        gidrel_tabs=gidrel_tabs, g_los=g_los,
    )


# ---------------------------------------------------------------- bass build
def _build_bass(plan, eps_vals):
    AL = mybir.AluOpType
    AF = mybir.ActivationFunctionType
    dt = mybir.dt
    f32, bf16 = dt.float32, dt.bfloat16

    NTILES = plan["NTILES"]
    BSTG = plan["BSTG"]

    nc = bacc.Bacc(trn_type="TRN2", num_devices=CORES, num_swdge_queues=2)
    groups = [list(range(CORES))]

    own0_e = nc.declare_dram_parameter("own0", [NPAD, D], bf16, False)
    gidx_e = nc.declare_dram_parameter("gidx", [128, NTILES], dt.int32, False)
    dstrel_e = nc.declare_dram_parameter("dstrel", [D, NTILES], bf16, False)
    gidrel_e = nc.declare_dram_parameter("gidrel", [D, TPC], bf16, False)
    iota_e = nc.declare_dram_parameter("iota", [D, GWIN], bf16, False)
    ident_e = nc.declare_dram_parameter("ident", [D, D], bf16, False)
    wt_e = nc.declare_dram_parameter("wt", [D, 3 * D], f32, False)
    cvec_e = nc.declare_dram_parameter("cvec", [D, 15], f32, False)
    out_e = nc.declare_dram_parameter("out", [D, GWIN], f32, True)

    table = nc.dram_tensor("table_i", [TROWS, D], bf16, addr_space="Shared")
    ownsl = nc.dram_tensor("ownsl", [NPAD, D], bf16)
    stin = [nc.dram_tensor(f"stin{l}", [D, 2], f32) for l in range(NLAYER)]
    stout = [
        nc.dram_tensor(f"stout{l}", [D, 2], f32, addr_space="Shared")
        for l in range(NLAYER)
    ]

    def onehot3(oh_tile, ncols, src_col0, ntab, iota_w):
        """oh[p, j*iota_w + n] = (n == ntab[p, src_col0 + j]) for j < ncols."""
        oa = oh_tile[:, : ncols * iota_w]
        oh3 = AP(oa.tensor, oa.offset, [oa.ap[0], [iota_w, ncols], [1, iota_w]])
        ia = iota_sb[:, :iota_w]
        io3 = AP(ia.tensor, ia.offset, [ia.ap[0], [0, ncols], [1, iota_w]])
        dr3 = ntab[:, src_col0 : src_col0 + ncols].to_broadcast(
            [D, ncols, iota_w]
        )
        nc.vector.tensor_tensor(out=oh3, in0=io3, in1=dr3, op=AL.is_equal)

    with TileContext(nc) as tc:
        with (
            tc.tile_pool(name="const", bufs=1) as cpool,
            tc.tile_pool(name="state", bufs=1) as spool,
            tc.tile_pool(name="stg", bufs=2) as stgp,
            tc.tile_pool(name="oh", bufs=2) as ohp,
            tc.tile_pool(name="oh2", bufs=2) as oh2p,
            tc.tile_pool(name="pre", bufs=6) as prep,
            tc.tile_pool(name="sq", bufs=4) as sqp,
            tc.tile_pool(name="hnm", bufs=4) as hnmp,
            tc.tile_pool(name="stat", bufs=2) as statp,
            tc.tile_pool(name="agg", bufs=4, space="PSUM") as aggp,
            tc.tile_pool(name="wmm", bufs=2, space="PSUM") as wmmp,
            tc.tile_pool(name="ptr", bufs=1, space="PSUM") as ptrp,
            tc.tile_pool(name="ppool", bufs=1, space="PSUM") as ppoolp,
        ):
            # ---- constants to SBUF
            gidx_sb = cpool.tile([128, NTILES], dt.int32, tag="gidx")
            dstrel_sb = cpool.tile([D, NTILES], bf16, tag="dstrel")
            gidrel_sb = cpool.tile([D, TPC], bf16, tag="gidrel")
            iota_sb = cpool.tile([D, GWIN], bf16, tag="iota")
            ident_sb = cpool.tile([D, D], bf16, tag="ident")
            wt_sb = cpool.tile([D, 3 * D], f32, tag="wt")
            cvec_sb = cpool.tile([D, 15], f32, tag="cvec")
            for t_, e_ in [
                (gidx_sb, gidx_e), (dstrel_sb, dstrel_e), (gidrel_sb, gidrel_e),
                (iota_sb, iota_e), (ident_sb, ident_e), (wt_sb, wt_e),
                (cvec_sb, cvec_e),
            ]:
                nc.sync.dma_start(out=t_[:, :], in_=e_[:, :])

            hT = spool.tile([D, NPAD], bf16, tag="hT")          # h^T bf16
            hlin = spool.tile([D, NPAD], f32, tag="hlin")       # linear out f32
            sumtab = spool.tile([D, TPC], f32, tag="sumtab")
            sqtab = spool.tile([D, TPC], f32, tag="sqtab")

            # initial state: hT via DMA transpose; table via AllGather
            nc.sync.dma_start(out=hT[:, :], in_=own0_e[:, :], transpose=True)
            nc.sync.dma_start(out=ownsl[:, :], in_=own0_e[:, :])
            nc.gpsimd.collective_compute(
                "AllGather", AL.bypass, replica_groups=groups,
                ins=[ownsl.ap().opt()], outs=[table.ap().opt()],
            )

            ppool_ps = ppoolp.tile([D, GWIN], mybir.dt.float32, tag="ppool")

            for l in range(NLAYER):
                for b in range(NB):
                    P0, nt, mlist = plan["binfo"][b]
                    stg = stgp.tile([D, BSTG], bf16, tag="stg")
                    for j in range(nt):
                        nc.gpsimd.indirect_dma_start(
                            out=stg[:, j * 128 : (j + 1) * 128],
                            out_offset=None,
                            in_=table[:, :],
                            in_offset=bass.IndirectOffsetOnAxis(
                                ap=gidx_sb[:, P0 + j : P0 + j + 1], axis=0
                            ),
                        )
                    oh = ohp.tile([D, BSTG], bf16, tag="oh")
                    onehot3(oh, nt, P0, dstrel_sb, 128)
                    for (t, plo, phi) in mlist:
                        ps = aggp.tile([D, 128], mybir.dt.float32, tag="agg")
                        for j in range(plo, phi):
                            nc.tensor.matmul(
                                out=ps[:, :],
                                lhsT=stg[:, (j - P0) * 128 : (j - P0 + 1) * 128],
                                rhs=oh[:, (j - P0) * 128 : (j - P0 + 1) * 128],
                                start=(j == plo), stop=(j == phi - 1),
                            )
                        pre = prep.tile([D, 128], mybir.dt.float32, tag="pre")
                        nc.vector.scalar_tensor_tensor(
                            out=pre[:, :], in0=hT[:, t * 128 : (t + 1) * 128],
                            scalar=float(1.0 + eps_vals[l]), in1=ps[:, :],
                            op0=AL.mult, op1=AL.add,
                        )
                        wps = wmmp.tile([D, 128], mybir.dt.float32, tag="wmm")
                        nc.tensor.matmul(
                            out=wps[:, :], lhsT=wt_sb[:, l * D : (l + 1) * D],
                            rhs=pre[:, :], start=True, stop=True,
                        )
                        nc.scalar.activation(
                            out=hlin[:, t * 128 : (t + 1) * 128], in_=wps[:, :],
                            func=AF.Identity, bias=cvec_sb[:, l : l + 1],
                            accum_out=sumtab[:, t : t + 1],
                        )
                        sqt = sqp.tile([D, 128], mybir.dt.float32, tag="sq")
                        nc.scalar.activation(
                            out=sqt[:, :], in_=hlin[:, t * 128 : (t + 1) * 128],
                            func=AF.Square, accum_out=sqtab[:, t : t + 1],
                        )

                # ---- batchnorm stats across all cores
                st = statp.tile([D, 16], mybir.dt.float32, tag="st")
                nc.vector.tensor_reduce(
                    out=st[:, 0:1], in_=sumtab[:, :TPC],
                    axis=mybir.AxisListType.X, op=AL.add,
                )
                nc.vector.tensor_reduce(
                    out=st[:, 1:2], in_=sqtab[:, :TPC],
                    axis=mybir.AxisListType.X, op=AL.add,
                )
                nc.sync.dma_start(out=stin[l][:, :], in_=st[:, 0:2])
                nc.gpsimd.collective_compute(
                    "AllReduce", AL.add, replica_groups=groups,
                    ins=[stin[l].ap().opt()], outs=[stout[l].ap().opt()],
                )
                nc.sync.dma_start(out=st[:, 2:4], in_=stout[l][:, :])
                # mean = gsum/N - sumfix ; ex2 = gsq/N - sqfix
                nc.vector.scalar_tensor_tensor(
                    out=st[:, 4:5], in0=st[:, 2:3], scalar=1.0 / N,
                    in1=cvec_sb[:, 3 + l : 4 + l], op0=AL.mult, op1=AL.subtract,
                )
                nc.vector.scalar_tensor_tensor(
                    out=st[:, 5:6], in0=st[:, 3:4], scalar=1.0 / N,
                    in1=cvec_sb[:, 6 + l : 7 + l], op0=AL.mult, op1=AL.subtract,
                )
                # m2 = mean^2 ; vareps = ex2 - m2 + eps ; inv = 1/sqrt(vareps)
                nc.vector.tensor_tensor(out=st[:, 6:7], in0=st[:, 4:5], in1=st[:, 4:5], op=AL.mult)
                nc.vector.tensor_scalar(
                    st[:, 7:8], st[:, 6:7], -1.0, BN_EPS, AL.mult, AL.add
                )
                nc.vector.tensor_tensor(out=st[:, 8:9], in0=st[:, 5:6], in1=st[:, 7:8], op=AL.add)
                nc.scalar.activation(out=st[:, 9:10], in_=st[:, 8:9], func=AF.Sqrt)
                nc.vector.reciprocal(out=st[:, 10:11], in_=st[:, 9:10])
                # s = gamma*inv ; c = beta - mean*s
                nc.vector.tensor_tensor(out=st[:, 11:12], in0=st[:, 10:11], in1=cvec_sb[:, 9 + l : 10 + l], op=AL.mult)
                nc.vector.tensor_tensor(out=st[:, 12:13], in0=st[:, 4:5], in1=st[:, 11:12], op=AL.mult)
                nc.vector.tensor_tensor(out=st[:, 13:14], in0=cvec_sb[:, 12 + l : 13 + l], in1=st[:, 12:13], op=AL.subtract)
                s_col = st[:, 11:12]
                c_col = st[:, 13:14]

                # ---- normalize + relu, then transpose to node-major
                for b in range(NB):
                    ts = list(range(b * TPB, min((b + 1) * TPB, TPC)))
                    ntl = len(ts)
                    if l == NLAYER - 1:
                        oh2 = oh2p.tile([D, TPB * GWIN], bf16, tag="oh2")
                        onehot3(oh2, ntl, ts[0], gidrel_sb, GWIN)
                    for i, t in enumerate(ts):
                        sl = slice(t * 128, (t + 1) * 128)
                        nc.scalar.activation(
                            out=hT[:, sl], in_=hlin[:, sl], func=AF.Relu,
                            scale=s_col, bias=c_col,
                        )
                        ptr = ptrp.tile([D, D], bf16, tag="ptr")
                        nc.tensor.transpose(ptr[:, :], hT[:, sl], ident_sb[:, :])
                        hnm = hnmp.tile([D, D], bf16, tag="hnm")
                        nc.vector.tensor_copy(out=hnm[:, :], in_=ptr[:, :])
                        if l < NLAYER - 1:
                            nc.sync.dma_start(
                                out=ownsl[t * 128 : (t + 1) * 128, :],
                                in_=hnm[:, :],
                            )
                        else:
                            nc.tensor.matmul(
                                out=ppool_ps[:, :], lhsT=hnm[:, :],
                                rhs=oh2[:, i * GWIN : (i + 1) * GWIN],
                                start=(t == 0), stop=(t == TPC - 1),
                            )
                if l < NLAYER - 1:
                    nc.vector.memset(hT[:, NLOC:NPAD], 0.0)
                    nc.gpsimd.collective_compute(
                        "AllGather", AL.bypass, replica_groups=groups,
                        ins=[ownsl.ap().opt()], outs=[table.ap().opt()],
                    )

            osb = statp.tile([D, GWIN], mybir.dt.float32, tag="osb")
            nc.vector.tensor_copy(out=osb[:, :], in_=ppool_ps[:, :])
            nc.sync.dma_start(out=out_e[:, :], in_=osb[:, :])

    nc.finalize()  # Bacc: runs the full compile pipeline (regs, event sems)
    return nc


# ---------------------------------------------------------------- entry
def _prep_inputs(x, W1, b1, W2, b2, W3, b3, gamma, beta, plan):
    x = np.asarray(x, np.float32)

    Ws = [np.asarray(w, np.float32) for w in (W1, W2, W3)]
    bs = [np.asarray(v, np.float32) for v in (b1, b2, b3)]
    gs = np.asarray(gamma, np.float32)
    be = np.asarray(beta, np.float32)
    wt = np.concatenate([w.T for w in Ws], axis=1).astype(np.float32).copy()
    npadc = float(CORES * (NPAD - NLOC))  # padded node count global
    cvec = np.zeros((D, 15), np.float32)
    for l in range(3):
        cvec[:, l] = bs[l]
        cvec[:, 3 + l] = bs[l] * (npadc / N)
        cvec[:, 6 + l] = (bs[l] ** 2) * (npadc / N)
        cvec[:, 9 + l] = gs[l]
        cvec[:, 12 + l] = be[l]

    iota = np.tile(np.arange(GWIN, dtype=np.float32), (D, 1)).astype(BF16)
    ident = np.eye(D, dtype=np.float32).astype(BF16)

    in_maps = []
    for c in range(CORES):
        own0 = np.zeros((NPAD, D), BF16)
        own0[:NLOC] = x[c * NLOC : (c + 1) * NLOC].astype(BF16)
        in_maps.append(
            dict(
                own0=own0,
                gidx=plan["gidx_tabs"][c],
                dstrel=plan["dstrel_tabs"][c],
                gidrel=plan["gidrel_tabs"][c],
                iota=iota,
                ident=ident,
                wt=wt,
                cvec=cvec,
            )
        )
    return in_maps


def _kernel_np(inputs):
    """Host fallback mirroring the reference in float32."""
    x = np.asarray(inputs["x"], np.float32)
    ei = np.asarray(inputs["edge_index"], np.int64)
    batch = np.asarray(inputs["batch"], np.int64)
    eps = np.asarray(inputs["eps"], np.float32)
    gamma = np.asarray(inputs["gamma"], np.float32)
    beta = np.asarray(inputs["beta"], np.float32)
    Ws = [np.asarray(inputs[k], np.float32) for k in ("W1", "W2", "W3")]
    bs = [np.asarray(inputs[k], np.float32) for k in ("b1", "b2", "b3")]
    src, dst = ei[0], ei[1]
    perm = np.argsort(dst, kind="stable")
    sdst = dst[perm]
    ssrc = src[perm]
    uniq, starts = np.unique(sdst, return_index=True)
    h = x
    for i in range(3):
        gathered = h[ssrc]
        agg = np.zeros_like(h)
        agg[uniq] = np.add.reduceat(gathered, starts, axis=0)
        h = (1.0 + eps[i]) * h + agg
        h = h @ Ws[i].T + bs[i]
        mean = h.mean(0)
        var = h.var(0)
        h = (h - mean) / np.sqrt(var + BN_EPS) * gamma[i] + beta[i]
        h = np.maximum(h, 0.0)
    sums = np.zeros((NGR, D), np.float32)
    bu, bstarts = np.unique(batch, return_index=True)
    sums[bu] = np.add.reduceat(h, bstarts, axis=0)
    cnt = np.bincount(batch, minlength=NGR).astype(np.float32)
    return sums / np.maximum(cnt, 1.0)[:, None]


def kernel(**inputs):
    import os
    if os.environ.get("GIN_FORCE_NUMPY"):
        return _kernel_np(inputs)
    for attempt in range(2):
        try:
            return _kernel_bass(inputs)
        except Exception as e:
            import traceback
            traceback.print_exc()
            print(f"bass path attempt {attempt} failed:", repr(e)[:200])
    return _kernel_np(inputs)


IN_NAMES = ["own0", "gidx", "dstrel", "gidrel", "iota", "ident", "wt", "cvec"]


def _run_pjrt_overlapped(nc, box):
    """Mirror of bass2jax.run_bass_via_pjrt's multi-core path, but taking
    pre-device_put global input arrays from `box` (shipped concurrently
    with the bass build)."""
    import jax
    from jax.sharding import PartitionSpec
    from jax.experimental.shard_map import shard_map
    from concourse import bass2jax

    bass2jax.install_neuronx_cc_hook()
    assert nc.dbg_addr is None

    partition_name = nc.partition_id_tensor.name if nc.partition_id_tensor else None
    in_names, out_names, out_avals = [], [], []
    for alloc in nc.m.functions[0].allocations:
        if not isinstance(alloc, mybir.MemoryLocationSet):
            continue
        name = alloc.memorylocations[0].name
        if alloc.kind == "ExternalInput":
            if name != partition_name:
                in_names.append(name)
        elif alloc.kind == "ExternalOutput":
            out_names.append(name)
            out_avals.append(
                jax.core.ShapedArray(
                    tuple(alloc.tensor_shape), mybir.dt.np(alloc.dtype)
                )
            )
    assert in_names == IN_NAMES, in_names
    assert out_names == ["out"], out_names
    n_params = len(in_names)
    all_names = in_names + out_names
    if partition_name is not None:
        all_names.append(partition_name)

    def _body(*args):
        operands = list(args)
        if partition_name is not None:
            operands.append(bass2jax.partition_id_tensor())
        outs = bass2jax._bass_exec_p.bind(
            *operands,
            out_avals=tuple(out_avals),
            in_names=tuple(all_names),
            out_names=tuple(out_names),
            lowering_input_output_aliases=(),
            sim_require_finite=True,
            sim_require_nnan=True,
            nc=nc,
        )
        return tuple(outs)

    mesh = box["mesh"]
    nio = n_params + len(out_names)
    sharded = jax.jit(
        shard_map(
            _body,
            mesh=mesh,
            in_specs=(PartitionSpec("core"),) * nio,
            out_specs=(PartitionSpec("core"),) * len(out_names),
            check_rep=False,
        ),
        donate_argnums=tuple(range(n_params, nio)),
        keep_unused=True,
    )
    out_arrs = sharded(*box["arrs"], box["zo"])
    return np.asarray(out_arrs[0]).reshape(CORES, D, GWIN)


def _kernel_bass(inputs):
    ei = np.asarray(inputs["edge_index"])
    batch = np.asarray(inputs["batch"])
    eps = np.asarray(inputs["eps"], np.float32)

    plan = _make_plan(ei[0], ei[1], batch)
    in_maps = _prep_inputs(
        inputs["x"], inputs["W1"], inputs["b1"], inputs["W2"], inputs["b2"],
        inputs["W3"], inputs["b3"], inputs["gamma"], inputs["beta"], plan,
    )

    import threading

    box = {}
    err = []

    def _ship():
        try:
            import jax
            from jax.sharding import Mesh, PartitionSpec, NamedSharding

            devices = jax.devices()[:CORES]
            mesh = Mesh(np.asarray(devices), ("core",))
            sh = NamedSharding(mesh, PartitionSpec("core"))
            arrs = []
            for name in IN_NAMES:
                g = np.concatenate(
                    [np.asarray(in_maps[c][name]) for c in range(CORES)], axis=0
                )
                arrs.append(jax.device_put(g, sh))
            zo = jax.device_put(np.zeros((CORES * D, GWIN), np.float32), sh)
            for a in arrs:
                a.block_until_ready()
            zo.block_until_ready()
            box["mesh"], box["arrs"], box["zo"] = mesh, arrs, zo
        except Exception as e:  # surface in main thread
            err.append(e)

    th = threading.Thread(target=_ship)
    th.start()
    nc = _build_bass(plan, [float(e) for e in eps])
    th.join()
    if err:
        raise err[0]

    out = _run_pjrt_overlapped(nc, box)  # [CORES, 128, GWIN]
    global LAST_RESULT
    LAST_RESULT = None

    full = np.zeros((NGR, D), np.float64)
    for c in range(CORES):
        g_lo = plan["g_los"][c]
        w = min(GWIN, NGR - g_lo)
        full[g_lo : g_lo + w] += out[c, :, :w].T
    counts = np.bincount(np.asarray(batch, np.int64), minlength=NGR).astype(np.float64)
    full /= np.maximum(counts, 1.0)[:, None]
    return full.astype(np.float32)
